# revision 1
# baseline (speedup 1.0000x reference)
"""Trainium2 Bass kernel for CenterAttentionLoss (v2).

Math: heat[b,p] = max_n exp(-d2(p, center_n)/(2*sigma^2)) over valid objects
(sigma=2 -> divisor 8), loss = mean((sigmoid(att)-heat)^2) * 0.05.

Single-scale power-mean: S[x,y] = sum_n exp(-16*dx2) * exp(-16*dy2) is a
K=128-chunked matmul over objects, and heat = S^(1/128) up to a tie factor
T^(1/128) ~ 1.0007 (corrected via the exp bias constant). The 128th root is
computed via the input-side log2 bit hack finished on the already-loaded Exp
table: heat = exp((float(bitcast_i32(S)) * ln2/2^23 - (127+sigma)*ln2)/128),
masked to 0 where S underflows f32 (true heat < ~0.5 there; P ~ 2e-4/px).
Single-seed rel loss error ~1e-3 (gate 2e-2).

Sharding: 8 cores = 4 batches x 2 y-halves. Objects are compacted on host
(cls>0, cy within the core's y-range +-4) and padded with far-away dummies
to NCH*128 slots; a y-shift is folded into cy so the device grid is always
0..31. Per core: 2 contiguous DMAs in, NCH matmuls accumulating S^T [64,32]
in PSUM, short DVE epilogue, fused MSE reduce, one scalar DMA out. Host
sums the 8 partials.

Only the Exp activation table is ever used (loaded once, warmed by a dummy
activation at t=0): sigmoid = reciprocal_approx_fast(1+exp(-att)) on DVE,
root finished through Exp as above.
"""

import math

import numpy as np

B, H, W = 4, 64, 64
HH = H // 2          # y rows per core
NCORES = 8
NCH = 10             # object chunks of 128 (1280 slots; max seen ~1140)
MARGIN = 2.0         # cy margin rows beyond the core's y-range
KSCALE = 16.0        # k/(2 sigma^2) with k=128
SIGMA_M = 0.045      # Mitchell log2-hack mantissa correction
SCALE_W = 0.05 / (B * H * W)
LN2 = math.log(2.0)
EXP_SCL = LN2 / (128.0 * 2.0**23)
EXP_BIAS = -(127.0 + SIGMA_M) * LN2 / 128.0
# mask threshold: compare float(bitcast_i32(S)) against bits of 1.2e-38
TH_BITS = float(np.frombuffer(np.float32(1.2e-38).tobytes(), np.int32)[0])

_cache: dict = {}
_GX = np.ascontiguousarray(
    np.broadcast_to(np.arange(W, dtype=np.float32), (128, W)))


def _build_program(nch):
    from contextlib import ExitStack

    import concourse.bacc as bacc
    import concourse.mybir as mybir
    import concourse.tile as tile

    f32 = mybir.dt.float32
    i32 = mybir.dt.int32
    bf16 = mybir.dt.bfloat16
    Alu = mybir.AluOpType
    Act = mybir.ActivationFunctionType

    ngrp = 2
    cpg = nch // ngrp

    nc = bacc.Bacc("TRN2", target_bir_lowering=False, debug=False)

    cxy_d = nc.dram_tensor("cxy", [128, 2 * nch], f32, kind="ExternalInput").ap()
    att_d = nc.dram_tensor("att", [W, HH], f32, kind="ExternalInput").ap()
    out_d = nc.dram_tensor("out", [1, 1], f32, kind="ExternalOutput").ap()

    with ExitStack() as ctx:
        tc = ctx.enter_context(tile.TileContext(nc))
        cpool = ctx.enter_context(tc.tile_pool(name="consts", bufs=1))
        wpool = ctx.enter_context(tc.tile_pool(name="work", bufs=ngrp))
        pspool = ctx.enter_context(tc.tile_pool(name="ps", bufs=1, space="PSUM"))
        epool = ctx.enter_context(tc.tile_pool(name="epi", bufs=1))

        # input DMAs first (latency), on two idle queues
        cxy = cpool.tile([128, 2 * nch], f32, tag="cxy")
        nc.sync.dma_start(out=cxy[:], in_=cxy_d)
        attS = cpool.tile([W, HH], f32, tag="attS")
        nc.scalar.dma_start(out=attS[:], in_=att_d)

        # grid coords on GPSIMD (no DMA): 0..63 i32, cast once on DVE
        gxi = cpool.tile([128, W], i32, tag="gxi")
        nc.gpsimd.iota(gxi[:], pattern=[[1, W]], channel_multiplier=0)
        gxF = cpool.tile([128, W], f32, tag="gxF")
        nc.vector.tensor_copy(out=gxF[:], in_=gxi[:])

        # warm the Exp table with a no-input-dep activation
        win = cpool.tile([1, 8], f32, tag="win")
        nc.vector.memset(win[:], 0.0)
        wout = cpool.tile([1, 8], f32, tag="wout")
        nc.scalar.activation(out=wout[:], in_=win[:], func=Act.Exp, scale=1.0)

        # per-partition bias vector for the root-finish Exp
        biasT = cpool.tile([128, 1], f32, tag="biasT")
        nc.vector.memset(biasT[:], EXP_BIAS)
        # ones column for the PE partition-sum of the MSE
        ones = cpool.tile([W, 1], f32, tag="ones")
        nc.vector.memset(ones[:], 1.0)

        # S^T accumulation: PS[x, y] = sum_n u[n,x] v[n,y]
        PS = pspool.tile([W, HH], f32, tag="PS", name="PS")

        bounds = [0, 6, nch]
        for g in range(ngrp):
            g0, g1 = bounds[g], bounds[g + 1]
            gn = g1 - g0
            cs = slice(g0, g1)
            cs2 = slice(nch + g0, nch + g1)
            shpU = [128, gn, W]
            shpV = [128, gn, HH]
            shpUV = [128, gn, W + HH]
            uvd = wpool.tile(shpUV, f32, tag="uvd")
            nc.vector.tensor_tensor(
                out=uvd[:, :, 0:W],
                in0=gxF[:].unsqueeze(1).broadcast_to(shpU),
                in1=cxy[:, cs].unsqueeze(2).broadcast_to(shpU),
                op=Alu.subtract,
            )
            nc.vector.tensor_tensor(
                out=uvd[:, :, W:W + HH],
                in0=gxF[:, 0:HH].unsqueeze(1).broadcast_to(shpV),
                in1=cxy[:, cs2].unsqueeze(2).broadcast_to(shpV),
                op=Alu.subtract,
            )
            uvsq = wpool.tile(shpUV, bf16, tag="uvsq")
            nc.vector.tensor_mul(out=uvsq[:], in0=uvd[:], in1=uvd[:])
            uvb = wpool.tile(shpUV, bf16, tag="uvb")
            nc.scalar.activation(
                out=uvb[:], in_=uvsq[:], func=Act.Exp, scale=-KSCALE)

            for cc in range(gn):
                ci = g0 + cc
                nc.tensor.matmul(
                    out=PS[:],
                    lhsT=uvb[:, cc, 0:W],
                    rhs=uvb[:, cc, W:W + HH],
                    start=(ci == 0),
                    stop=(ci == nch - 1),
                    skip_group_check=True,
                )

        # sigmoid(att) = 1/(1+exp(-att)); overlaps with matmul phase
        eA = epool.tile([W, HH], f32, tag="eA")
        nc.scalar.activation(out=eA[:], in_=attS[:], func=Act.Exp, scale=-1.0)
        tA = epool.tile([W, HH], f32, tag="tA")
        nc.vector.tensor_scalar(
            out=tA[:], in0=eA[:], scalar1=1.0, scalar2=None, op0=Alu.add)
        sg = epool.tile([W, HH], f32, tag="sg")
        nc.vector.reciprocal(out=sg[:], in_=tA[:])

        # heat = Exp(float(bitcast_i32(S)) * scl + bias), masked where S
        # underflowed (int bits below those of 1.2e-38)
        eif = epool.tile([W, HH], f32, tag="eif")
        nc.vector.tensor_copy(out=eif[:], in_=PS[:].bitcast(i32))
        r = epool.tile([W, HH], f32, tag="r")
        nc.scalar.activation(
            out=r[:], in_=eif[:], func=Act.Exp, scale=EXP_SCL, bias=biasT[0:W])
        heat = epool.tile([W, HH], f32, tag="heat")
        nc.vector.scalar_tensor_tensor(
            out=heat[:], in0=eif[:], scalar=TH_BITS, in1=r[:],
            op0=Alu.is_gt, op1=Alu.mult,
        )
        diff = epool.tile([W, HH], f32, tag="diff")
        nc.vector.tensor_sub(out=diff[:], in0=sg[:], in1=heat[:])
        scr = epool.tile([W, HH], f32, tag="scr")
        nc.vector.tensor_mul(out=scr[:], in0=diff[:], in1=diff[:])
        colsum = epool.tile([W, 1], f32, tag="colsum")
        nc.vector.tensor_reduce(
            out=colsum[:], in_=scr[:], axis=mybir.AxisListType.X, op=Alu.add)
        # partition-sum via PE; host applies the final scale
        PT = pspool.tile([1, 1], f32, tag="PT", name="PT")
        nc.tensor.matmul(
            out=PT[:], lhsT=colsum[:], rhs=ones[:],
            start=True, stop=True, skip_group_check=True,
        )
        fin = epool.tile([1, 1], f32, tag="fin")
        nc.vector.tensor_copy(out=fin[:], in_=PT[:])
        nc.sync.dma_start(out=out_d, in_=fin[:])

    nc.compile()
    return nc


def _get_program(nch=NCH):
    if nch not in _cache:
        _cache[nch] = _build_program(nch)
    return _cache[nch]


def _pack_inputs(att, cls_t, box, nch):
    """Per-core compacted/padded inputs; returns (in_maps, max_count)."""
    in_maps = []
    maxn = 0
    cap = nch * 128
    for c in range(NCORES):
        b, hh = c % B, c // B
        sel = cls_t[b].reshape(-1) > 0
        bx = box[b].reshape(-1, 2)
        cx_all = bx[sel, 0]
        cy_all = bx[sel, 1]
        lo, hi = HH * hh - MARGIN, HH * hh + HH + MARGIN
        m = (cy_all >= lo) & (cy_all < hi)
        cx = cx_all[m]
        cy = cy_all[m] - np.float32(HH * hh)
        n = cx.size
        maxn = max(maxn, n)
        if n > cap:
            return None, maxn
        cxp = np.full(cap, 1.0e4, np.float32)
        cxp[:n] = cx
        cyp = np.full(cap, 1.0e4, np.float32)
        cyp[:n] = cy
        cxy = np.empty((128, 2 * nch), np.float32)
        cxy[:, :nch] = cxp.reshape(nch, 128).T
        cxy[:, nch:] = cyp.reshape(nch, 128).T
        attT = np.ascontiguousarray(
            att[b, 0, HH * hh: HH * (hh + 1), :].T)  # [W, HH]
        in_maps.append({"cxy": cxy, "att": attT})
    return in_maps, maxn


def kernel(attention_maps, class_targets, box_targets):
    from concourse.bass_utils import run_bass_kernel_spmd

    att = np.ascontiguousarray(np.asarray(attention_maps, dtype=np.float32))
    cls_t = np.ascontiguousarray(np.asarray(class_targets, dtype=np.int32))
    box = np.ascontiguousarray(np.asarray(box_targets, dtype=np.float32))

    nch = NCH
    in_maps, maxn = _pack_inputs(att, cls_t, box, nch)
    if in_maps is None:  # statistically impossible overflow; recompile wider
        nch = (maxn + 127) // 128
        in_maps, _ = _pack_inputs(att, cls_t, box, nch)
    nc = _get_program(nch)
    res = run_bass_kernel_spmd(nc, in_maps, list(range(NCORES))).results
    total = np.float32(0.0)
    for c in range(NCORES):
        total = total + np.float32(res[c]["out"].sum(dtype=np.float32))
    return np.asarray(np.float32(total * np.float32(SCALE_W)), dtype=np.float32)



# revision 2
# speedup vs baseline: 1.0419x; 1.0419x over previous
"""Trainium2 Bass kernel for CenterAttentionLoss (v3).

Math: heat[b,p] = max_n exp(-d2(p, center_n)/(2*sigma^2)) over valid objects
(sigma=2 -> divisor 8), loss = mean((sigmoid(att)-heat)^2) * 0.05.

Power-mean with k=128: S[x,y] = sum_n exp(-16*dx2)*exp(-16*dy2) accumulated
as NCH K=128 matmuls in PSUM; heat = S^(1/128) up to a ~1.0007 tie factor
(folded into the exp bias). The 128th root reads the f32 bit pattern of S
directly: r = Exp(float(bitcast_i32(S)) * ln2/2^23/128 - (127+sigma_m)*ln2/128)
on the Scalar engine (int32 input is value-converted). No underflow mask:
where S flushes below f32 range the true heat is < ~0.5 and r saturates at
~0.5; measured loss impact ~1e-4 relative (gate 2e-2).

MSE is expanded as sum(sg^2) - 2*sum(sg*r) + sum(r^2) and produced as three
[64,1] per-partition accumulator columns via the fused accum_out of one
scalar_tensor_tensor (DVE) and two Square activations (ACT) - no reduce, no
PE partition-sum, no final copy. Host sums 8 cores x 64 rows x 3 cols.

Sharding: 8 cores = 4 batches x 2 y-halves. Objects compacted on host
(cls>0, cy in range +-2) into fp16 chunks of 128, padded with cx=cy=200
dummies (d2 <= 40000 stays finite in fp16; exp -> 0). Coordinates, grid and
att ship as fp16: subtract/square run on DVE at 2x 16-bit rate; Gaussians
exp to bf16 on ACT for the PE matmuls. Two coordinate DMAs (first group
first) overlap the per-group DVE/ACT/PE pipeline startup; att DMA rides the
Scalar queue ahead of its activation-table load.
"""

import math

import numpy as np

B, H, W = 4, 64, 64
HH = H // 2          # y rows per core
NCORES = 8
NCH = 10             # object chunks of 128 (1280 slots; max seen ~1140)
MARGIN = 2.0         # cy margin rows beyond the core's y-range
KSCALE = 16.0        # k/(2 sigma^2) with k=128
SIGMA_M = 0.045      # Mitchell log2-hack mantissa correction
SCALE_W = 0.05 / (B * H * W)
DUMMY = 200.0        # pad center: (0-200)^2 = 40000 < fp16 max
LN2 = math.log(2.0)
EXP_SCL = LN2 / (128.0 * 2.0**23)
EXP_BIAS = -(127.0 + SIGMA_M) * LN2 / 128.0
ROOT_DIRECT = True   # Scalar ACT reads PSUM bits as int32 directly

_cache: dict = {}


def _group_bounds(nch):
    bs = sorted({0, min(4, nch), min(8, nch), nch})
    return [(bs[i], bs[i + 1]) for i in range(len(bs) - 1)]


def _build_program(nch):
    from contextlib import ExitStack

    import concourse.bacc as bacc
    import concourse.mybir as mybir
    import concourse.tile as tile

    f32 = mybir.dt.float32
    f16 = mybir.dt.float16
    i32 = mybir.dt.int32
    bf16 = mybir.dt.bfloat16
    Alu = mybir.AluOpType
    Act = mybir.ActivationFunctionType

    groups = _group_bounds(nch)
    g0n = groups[0][1] - groups[0][0]
    restn = nch - g0n

    nc = bacc.Bacc("TRN2", target_bir_lowering=False, debug=False)

    cxyA_d = nc.dram_tensor("cxyA", [128, 2 * g0n], f16, kind="ExternalInput").ap()
    if restn:
        cxyB_d = nc.dram_tensor("cxyB", [128, 2 * restn], f16,
                                kind="ExternalInput").ap()
    att_d = nc.dram_tensor("att", [W, HH], f16, kind="ExternalInput").ap()
    out_d = nc.dram_tensor("out", [W, 3], f32, kind="ExternalOutput").ap()

    with ExitStack() as ctx:
        tc = ctx.enter_context(tile.TileContext(nc))
        cpool = ctx.enter_context(tc.tile_pool(name="consts", bufs=1))
        wpool = ctx.enter_context(tc.tile_pool(name="work", bufs=len(groups)))
        pspool = ctx.enter_context(tc.tile_pool(name="ps", bufs=1, space="PSUM"))
        epool = ctx.enter_context(tc.tile_pool(name="epi", bufs=1))

        # input DMAs first (latency). sync: group coords in consumption
        # order; scalar: att (its act-table load follows the trigger).
        cxyA = cpool.tile([128, 2 * g0n], f16, tag="cxyA")
        nc.sync.dma_start(out=cxyA[:], in_=cxyA_d)
        if restn:
            cxyB = cpool.tile([128, 2 * restn], f16, tag="cxyB")
            nc.sync.dma_start(out=cxyB[:], in_=cxyB_d)
        attS = cpool.tile([W, HH], f16, tag="attS")
        nc.scalar.dma_start(out=attS[:], in_=att_d)

        # grid coords: iota on GPSIMD, cast to fp16 on DVE
        gxi = cpool.tile([128, W], i32, tag="gxi")
        nc.gpsimd.iota(gxi[:], pattern=[[1, W]], channel_multiplier=0)
        gxF = cpool.tile([128, W], f16, tag="gxF")
        nc.vector.tensor_copy(out=gxF[:], in_=gxi[:])

        # per-partition bias for the root-finish Exp
        biasT = cpool.tile([W, 1], f32, tag="biasT")
        nc.gpsimd.memset(biasT[:], EXP_BIAS)

        # sigmoid: eA = exp(-att) early on Scalar (after table load)
        eA = epool.tile([W, HH], f32, tag="eA")
        nc.scalar.activation(out=eA[:], in_=attS[:], func=Act.Exp, scale=-1.0)

        # S^T accumulation: PS[x, y] = sum_n u[n,x] v[n,y]
        PS = pspool.tile([W, HH], f32, tag="PS", name="PS")

        for g, (g0, g1) in enumerate(groups):
            gn = g1 - g0
            src = cxyA if g == 0 else cxyB
            off = 0 if g == 0 else 2 * (g0 - g0n)
            shpU = [128, gn, W]
            shpV = [128, gn, HH]
            shpUV = [128, gn, W + HH]
            uvd = wpool.tile(shpUV, f16, tag="uvd")
            nc.vector.tensor_tensor(
                out=uvd[:, :, 0:W],
                in0=gxF[:].unsqueeze(1).broadcast_to(shpU),
                in1=src[:, off:off + gn].unsqueeze(2).broadcast_to(shpU),
                op=Alu.subtract,
            )
            nc.vector.tensor_tensor(
                out=uvd[:, :, W:W + HH],
                in0=gxF[:, 0:HH].unsqueeze(1).broadcast_to(shpV),
                in1=src[:, off + gn:off + 2 * gn].unsqueeze(2).broadcast_to(shpV),
                op=Alu.subtract,
            )
            uvsq = wpool.tile(shpUV, f16, tag="uvsq")
            nc.vector.tensor_mul(out=uvsq[:], in0=uvd[:], in1=uvd[:])
            uvb = wpool.tile(shpUV, bf16, tag="uvb")
            nc.scalar.activation(
                out=uvb[:], in_=uvsq[:], func=Act.Exp, scale=-KSCALE)

            for cc in range(gn):
                ci = g0 + cc
                nc.tensor.matmul(
                    out=PS[:],
                    lhsT=uvb[:, cc, 0:W],
                    rhs=uvb[:, cc, W:W + HH],
                    start=(ci == 0),
                    stop=(ci == nch - 1),
                    skip_group_check=True,
                )

        # sigmoid finish on DVE (after the chunk subtract/squares)
        tA = epool.tile([W, HH], f32, tag="tA")
        nc.vector.tensor_scalar(
            out=tA[:], in0=eA[:], scalar1=1.0, scalar2=None, op0=Alu.add)
        sg = epool.tile([W, HH], f32, tag="sg")
        nc.vector.reciprocal(out=sg[:], in_=tA[:])

        # MSE partials: msep[:,0]=sum(sg^2)  msep[:,1]=-2*sum(sg*r)
        # msep[:,2]=sum(r^2); host sums all three columns.
        msep = epool.tile([W, 3], f32, tag="msep")
        jnk0 = epool.tile([W, HH], bf16, tag="jnk0")
        nc.scalar.activation(
            out=jnk0[:], in_=sg[:], func=Act.Square,
            accum_out=msep[:, 0:1])

        # 128th root: r = Exp(scl*float(bits(S)) + bias)
        r = epool.tile([W, HH], f32, tag="r")
        if ROOT_DIRECT:
            nc.scalar.activation(
                out=r[:], in_=PS[:].bitcast(i32), func=Act.Exp,
                scale=EXP_SCL, bias=biasT[:])
        else:
            eif = epool.tile([W, HH], f32, tag="eif")
            nc.vector.tensor_copy(out=eif[:], in_=PS[:].bitcast(i32))
            nc.scalar.activation(
                out=r[:], in_=eif[:], func=Act.Exp,
                scale=EXP_SCL, bias=biasT[:])

        jnk1 = epool.tile([W, HH], f32, tag="jnk1")
        nc.vector.scalar_tensor_tensor(
            out=jnk1[:], in0=r[:], scalar=-2.0, in1=sg[:],
            op0=Alu.mult, op1=Alu.mult, accum_out=msep[:, 1:2])
        jnk2 = epool.tile([W, HH], bf16, tag="jnk2")
        nc.scalar.activation(
            out=jnk2[:], in_=r[:], func=Act.Square,
            accum_out=msep[:, 2:3])

        nc.sync.dma_start(out=out_d, in_=msep[:])

    nc.compile()
    return nc


def _get_program(nch=NCH):
    if nch not in _cache:
        _cache[nch] = _build_program(nch)
    return _cache[nch]


def _pack_inputs(att, cls_t, box, nch):
    """Per-core compacted/padded fp16 inputs; returns (in_maps, max_count)."""
    groups = _group_bounds(nch)
    g0n = groups[0][1] - groups[0][0]
    in_maps = []
    maxn = 0
    cap = nch * 128
    for c in range(NCORES):
        b, hh = c % B, c // B
        sel = cls_t[b].reshape(-1) > 0
        bx = box[b].reshape(-1, 2)
        cx_all = bx[sel, 0]
        cy_all = bx[sel, 1]
        lo, hi = HH * hh - MARGIN, HH * hh + HH + MARGIN
        m = (cy_all >= lo) & (cy_all < hi)
        cx = cx_all[m]
        cy = cy_all[m] - np.float32(HH * hh)
        n = cx.size
        maxn = max(maxn, n)
        if n > cap:
            return None, maxn
        cxp = np.full(cap, DUMMY, np.float16)
        cxp[:n] = cx.astype(np.float16)
        cyp = np.full(cap, DUMMY, np.float16)
        cyp[:n] = cy.astype(np.float16)
        cxc = cxp.reshape(nch, 128).T  # [128, nch]
        cyc = cyp.reshape(nch, 128).T
        parts = {}
        for g, (a, bnd) in enumerate(groups):
            cols = np.concatenate([cxc[:, a:bnd], cyc[:, a:bnd]], axis=1)
            parts[g] = cols
        cxyA = np.ascontiguousarray(parts[0])
        im = {"cxyA": cxyA}
        if nch > g0n:
            im["cxyB"] = np.ascontiguousarray(
                np.concatenate([parts[g] for g in range(1, len(groups))], axis=1))
        attT = np.ascontiguousarray(
            att[b, 0, HH * hh: HH * (hh + 1), :].T.astype(np.float16))  # [W, HH]
        im["att"] = attT
        in_maps.append(im)
    return in_maps, maxn


def kernel(attention_maps, class_targets, box_targets):
    from concourse.bass_utils import run_bass_kernel_spmd

    att = np.ascontiguousarray(np.asarray(attention_maps, dtype=np.float32))
    cls_t = np.ascontiguousarray(np.asarray(class_targets, dtype=np.int32))
    box = np.ascontiguousarray(np.asarray(box_targets, dtype=np.float32))

    nch = NCH
    in_maps, maxn = _pack_inputs(att, cls_t, box, nch)
    if in_maps is None:  # statistically impossible overflow; recompile wider
        nch = (maxn + 127) // 128
        in_maps, _ = _pack_inputs(att, cls_t, box, nch)
    nc = _get_program(nch)
    res = run_bass_kernel_spmd(nc, in_maps, list(range(NCORES))).results
    total = np.float32(0.0)
    for c in range(NCORES):
        total = total + np.float32(res[c]["out"].sum(dtype=np.float32))
    return np.asarray(np.float32(total * np.float32(SCALE_W)), dtype=np.float32)
